# revision 1
# baseline (speedup 1.0000x reference)
"""BlockSparseLinear on 8 TRN2 NeuronCores.

Computes out = x @ W_dense.T + bias where W_dense is a [4096, 4096] matrix
assembled from 8192 nonzero 32x32 blocks (50% density).

Strategy:
  - Host: scatter the nonzero blocks into dense per-core weight shards, in the
    exact transposed/tiled DRAM layout the device kernel wants.
  - Sharding: 4-way over tokens x 2-way over out-features (8 cores).
    Per core: out_shard[1024 tokens, 2048 outf] = x_shard @ W_half.T + bias.
  - Device: dense matmul in float32r (FP22 reduced-precision fp32, full PE
    rate at moving-dim >= 256), out^T orientation (psum[o-part, token-free],
    stationary = weight tile, moving = x tile).
  - Phases iterate over k-blocks (4 contraction tiles each, 8 phases)
    sweeping ALL 16 o-tiles, with per-o-tile fp32 SBUF accumulators. This
    spreads the x and W HBM traffic evenly (~6MB per ~29us phase) and keeps
    the phase-0 critical first-DMA path short (2MB of x + first W tile). Bias is folded into the first
    phase's PSUM->SBUF accumulate; x DMAs ride the ACT HWDGE ring, W DMAs the
    SP ring, so neither queues behind the other.

Per-core loop nest:
  for kb in 8 (k-blocks of 4 k-tiles):
    load x[k] as per-(k, 512-token-chunk) tiles   (ACT ring, 256KB DMAs)
    for ot in 16 (o-tiles of 128 outf):
      load W[kb, ot] = [128k, 4, 128o]            (SP ring, one 256KB DMA)
      psum[tc] for tc in 2                  (2 PSUM banks, pipelined over ot)
      for k in kb, tc in 2:
        matmul(psum[tc], lhsT=w[kb,ot,k], rhs=x[k][tc], start/stop at kb edges)
      acc[ot] (+)= psum (+ bias at kb==0)   (DVE)
      if kb == 7: DMA acc[ot] -> out^T      (ACT ring; keep off the W ring)

Measured on the 8-core axon TRN2 pod: ~255-258us HW exec (slowest core), vs a
~232us pure-matmul floor (1024 MMs x 227ns) + fixed preamble/drain overheads.
Relative error vs the fp32 reference: 1.47e-4 (FP22 truncation).
"""

import os

import numpy as np

import concourse.mybir as mybir
import concourse.tile as tile
from concourse import bacc
from concourse.bass_utils import run_bass_kernel_spmd

BLOCK = 32
IN_FEATURES = 4096
OUT_FEATURES = 4096
N_TOKENS = 4096
IN_BLOCKS = IN_FEATURES // BLOCK  # 128
OUT_BLOCKS = OUT_FEATURES // BLOCK  # 128

N_CORES = 8
T_SHARDS = 4  # token shards
O_SHARDS = 2  # out-feature shards
TSH = N_TOKENS // T_SHARDS  # 1024 tokens per core
OSH = OUT_FEATURES // O_SHARDS  # 2048 out features per core

P = 128  # partitions
NFREE = 512  # matmul moving free dim (one PSUM bank of fp32)
K_TILES = IN_FEATURES // P  # 32
T_CHUNKS = TSH // NFREE  # 2 moving token chunks per core
O_TILES = OSH // P  # 16 o-tiles of 128 outf
KB_GROUPS = 8  # k-block phases
KB_SIZE = K_TILES // KB_GROUPS  # 8 k-tiles per phase

# exec time of the slowest core from the last traced run (ns), None if untraced
LAST_EXEC_NS = None
LAST_RESULT = None


def _install_axon_ntff_hook():
    """Best-effort: register the axon NTFF profiling hook that the image's
    antenv package lacks. Returns True if tracing is possible."""
    try:
        from antenv.axon_hooks import get_axon_ntff_profile_hook

        return get_axon_ntff_profile_hook() is not None
    except ImportError:
        pass
    try:
        import sys
        import types

        import antenv
        import trn_agent_boot.trn_boot as tb

        hook = tb._ntff_profile_via_ctypes("/opt/axon/libaxon_pjrt.so")
        if hook is None:
            return False
        mod = types.ModuleType("antenv.axon_hooks")
        mod._hook = hook
        mod.get_axon_ntff_profile_hook = lambda: mod._hook
        mod.set_axon_ntff_profile_hook = lambda h: setattr(mod, "_hook", h)
        sys.modules["antenv.axon_hooks"] = mod
        antenv.axon_hooks = mod

        # avoid the artifact-upload dependency in the trace path
        import concourse.bass_utils as bu

        bu.upload_artifacts = lambda tmpdir: str(tmpdir)
        return True
    except Exception:
        return False


def _build_bass():
    nc = bacc.Bacc(None, target_bir_lowering=False)

    x_d = nc.dram_tensor(
        "xt", [P, K_TILES, TSH], mybir.dt.float32r, kind="ExternalInput"
    )
    # wt[kb, ot, p(k), k4, o] = W[o0 + ot*128 + o, (kb*KB_SIZE + k4)*128 + p]
    w_d = nc.dram_tensor(
        "wt",
        [KB_GROUPS, O_TILES, P, KB_SIZE, P],
        mybir.dt.float32r,
        kind="ExternalInput",
    )
    b_d = nc.dram_tensor("bias", [P, O_TILES], mybir.dt.float32, kind="ExternalInput")
    o_d = nc.dram_tensor(
        "out", [O_TILES, P, TSH], mybir.dt.float32, kind="ExternalOutput"
    )

    with tile.TileContext(nc) as tc:
        with (
            tc.tile_pool(name="xpool", bufs=6 * KB_SIZE) as xpool,
            tc.tile_pool(name="wpool", bufs=16) as wpool,
            tc.tile_pool(name="apool", bufs=1) as apool,
            tc.tile_pool(name="bpool", bufs=1) as bpool,
            tc.tile_pool(name="psum", bufs=8, space="PSUM") as ppool,
        ):
            bias_sb = bpool.tile([P, O_TILES], mybir.dt.float32)

            acc_tiles = [
                apool.tile([P, TSH], mybir.dt.float32, tag=f"a{ot}", name="acc")
                for ot in range(O_TILES)
            ]

            for kb in range(KB_GROUPS):
                x_tiles = []
                for k8 in range(KB_SIZE):
                    k = kb * KB_SIZE + k8
                    row = []
                    for tcn in range(T_CHUNKS):
                        x_k = xpool.tile([P, NFREE], mybir.dt.float32r, tag="x", name="x")
                        nc.scalar.dma_start(
                            x_k[:], x_d[:, k, tcn * NFREE : (tcn + 1) * NFREE]
                        )
                        row.append(x_k)
                    x_tiles.append(row)
                if kb == 0:
                    nc.scalar.dma_start(bias_sb[:], b_d[:])

                for ot in range(O_TILES):
                    w_sb = wpool.tile(
                        [P, KB_SIZE, P], mybir.dt.float32r, tag="w", name="w"
                    )
                    nc.sync.dma_start(w_sb[:], w_d[kb, ot])
                    psums = [
                        ppool.tile([P, NFREE], mybir.dt.float32, tag="acc", name="ps")
                        for _ in range(T_CHUNKS)
                    ]
                    for k8 in range(KB_SIZE):
                        for tcn in range(T_CHUNKS):
                            nc.tensor.matmul(
                                psums[tcn][:],
                                lhsT=w_sb[:, k8],
                                rhs=x_tiles[k8][tcn][:],
                                start=(k8 == 0),
                                stop=(k8 == KB_SIZE - 1),
                            )
                    acc = acc_tiles[ot]
                    for tcn in range(T_CHUNKS):
                        sl = slice(tcn * NFREE, (tcn + 1) * NFREE)
                        if kb == 0:
                            nc.vector.tensor_tensor(
                                acc[:, sl],
                                psums[tcn][:],
                                bias_sb[:, ot : ot + 1].to_broadcast([P, NFREE]),
                                mybir.AluOpType.add,
                            )
                        else:
                            nc.vector.tensor_tensor(
                                acc[:, sl],
                                psums[tcn][:],
                                acc[:, sl],
                                mybir.AluOpType.add,
                            )
                        if kb == KB_GROUPS - 1:
                            nc.scalar.dma_start(o_d[ot, :, sl], acc[:, sl])

    nc.compile()
    return nc


def _dense_weight(weight_data, block_ids):
    """Scatter nonzero 32x32 blocks into dense [OUT, IN] (numpy, host-side)."""
    w = np.zeros((OUT_FEATURES, IN_FEATURES), dtype=np.float32)
    br = block_ids.astype(np.int64) // IN_BLOCKS
    bc = block_ids.astype(np.int64) % IN_BLOCKS
    # view as [OUT_BLOCKS, 32, IN_BLOCKS, 32] and scatter per-block
    w4 = w.reshape(OUT_BLOCKS, BLOCK, IN_BLOCKS, BLOCK)
    w4[br, :, bc, :] = weight_data
    return w


def kernel(x, weight_data, bias, block_ids):
    x = np.ascontiguousarray(np.asarray(x, dtype=np.float32))
    weight_data = np.asarray(weight_data, dtype=np.float32)
    bias = np.asarray(bias, dtype=np.float32)
    block_ids = np.asarray(block_ids)

    w = _dense_weight(weight_data, block_ids)  # [OUT, IN]

    # per-token-shard x^T in device layout [P, K_TILES, TSH]:
    # xt[p, k, t] = x[t0 + t, k*128 + p]
    xts = []
    for ti in range(T_SHARDS):
        xs = x[ti * TSH : (ti + 1) * TSH, :]  # [TSH, IN]
        xt = np.ascontiguousarray(
            xs.T.reshape(K_TILES, P, TSH).transpose(1, 0, 2)
        )  # [P, K_TILES, TSH]
        xts.append(xt)

    # per-outf-shard W in device layout [KB_GROUPS, O_TILES, P(k), KB_SIZE, P(o)]:
    # wt[kb, ot, p, k8, o] = W[o0 + ot*128 + o, (kb*8 + k8)*128 + p]
    wts = []
    biases = []
    for si in range(O_SHARDS):
        ws = w[si * OSH : (si + 1) * OSH, :]  # [OSH, IN]
        # [ot, o, kb, k4, p] -> [kb, ot, p, k4, o]
        wt = ws.reshape(O_TILES, P, KB_GROUPS, KB_SIZE, P).transpose(2, 0, 4, 3, 1)
        wts.append(np.ascontiguousarray(wt))
        bs = bias[si * OSH : (si + 1) * OSH]  # [OSH]
        biases.append(np.ascontiguousarray(bs.reshape(O_TILES, P).T))  # [P, O_TILES]

    in_maps = []
    for c in range(N_CORES):
        ti, si = c // O_SHARDS, c % O_SHARDS
        in_maps.append({"xt": xts[ti], "wt": wts[si], "bias": biases[si]})

    nc = _build_bass()
    trace = bool(int(os.environ.get("BSL_TRACE", "0")))
    if trace:
        trace = _install_axon_ntff_hook()
    kwargs = {}
    if trace:
        tdir = os.environ.get("BSL_TRACE_DIR")
        if tdir:
            os.makedirs(tdir, exist_ok=True)
            kwargs["tmpdir"] = tdir
        kwargs["trace_cores"] = list(range(N_CORES))
    res = run_bass_kernel_spmd(
        nc,
        in_maps,
        core_ids=list(range(N_CORES)),
        trace=trace,
        **kwargs,
    )

    global LAST_EXEC_NS, LAST_RESULT
    LAST_EXEC_NS = res.exec_time_ns
    LAST_RESULT = res

    out = np.empty((N_TOKENS, OUT_FEATURES), dtype=np.float32)
    for c in range(N_CORES):
        ti, si = c // O_SHARDS, c % O_SHARDS
        o = res.results[c]["out"]  # [O_TILES, P(o), TSH(t)]
        out[ti * TSH : (ti + 1) * TSH, si * OSH : (si + 1) * OSH] = o.reshape(
            OSH, TSH
        ).T
    return out

